# revision 21
# baseline (speedup 1.0000x reference)
"""Trainium2 Bass kernel for nn_AttentionHelper (sparse_attention) — v8.

Math (per batch b):
    nt[m,l] = exp(E[m,l]/16 + 2*ln(mask[m]+eps))   (= mask^2 * exp(E/16))
    d[l]    = sum_m w1m[m] * nt[m,l],  w1m = (mask+1e-9)/(mask+eps)^2
    out     = sum_m V[c,m] * nt[m,l] / d[l]

HW-informed design (microbenchmarked): bf16 matmuls run at ~107ns per
[128x128]x[128x512] (dual-pumped) when the PE stays continuously busy;
fp8 DoubleRow measured SLOWER (144ns) than bf16 on HW, so everything
stays bf16.  The design goal is keeping the PE stream gap-free so the
p-state stays at max:
  - denominator on the PE: d_ps[p,l] += w1rep[:,j,:]^T @ nt[:,j,:] with
    w1rep a broadcast-replicated weight column (reduces over m AND
    broadcasts d to all partitions in one op).
  - mask^2 folded into the exp's per-partition BIAS (2*ln(mask+eps)),
    computed once per rep by a single Ln activation over both batches'
    masks — removes the 16 post-transpose vt-fold DVE muls whose
    transpose-gated wait serialized the DVE queue at rep boundaries.
  - per-j PE order QK -> AV(prev) -> d(j-1): the d-matmul reads n1(j-1)
    which only clears the exp ~678ns after QK(j) starts; AV first covers
    that latency, d never stalls the PE.
  - ALL f32->bf16 conversions (q,k,v) on Pool/gpsimd: the DVE queue
    keeps only short, early-ready ops (w1rep broadcasts, normalize muls,
    reciprocal) so fin/rec are never head-of-line blocked by load-gated
    conversions.
  - loads V-first + vb converted first, so the vt transposes (SP queue)
    never block the next batch's loads.
  - rec emitted immediately after each jloop: d_ps (bufs=1) frees before
    the next jloop's first d-matmul needs the bank.
  - AV runs cg-sequential (cg0 done mid-loop, normalized+stored while
    cg1 accumulates) so PSUM fits: e_ps 2x2 + o_ps 1x2 + d_ps 1x2 = 8.

Pipeline: flat (rep, batch, half) jobs; prev carried across batch AND rep
boundaries; all DMA (loads, stores, transposes) on the SP HWDGE queue.
"""

import numpy as np

import concourse.bacc as bacc
import concourse.bass as bass
import concourse.tile as tile
from concourse import mybir
from concourse.bass_utils import run_bass_kernel_spmd

B, C, L = 16, 256, 2048
NCORES = 8
BS = B // NCORES
P = 128
CCH = C // P
MCH = L // P
NH = 2
LH = L // NH
LT = 512
F32 = mybir.dt.float32
BF16 = mybir.dt.bfloat16
import os as _os

EXP = mybir.ActivationFunctionType.Exp
LN = mybir.ActivationFunctionType.Ln
MEPS = 3e-5
# A/B toggles (HW-measured): exp-bias mask fold vs DVE vt-fold muls
EXP_BIAS = _os.environ.get("BASS_EXP_BIAS", "0") == "1"


def _emit(ctx, tc, q_d, k_d, v_d, m_d, o_d):
    nc = tc.nc

    qk_pool = ctx.enter_context(tc.tile_pool(name="qk", bufs=2))
    stage_pool = ctx.enter_context(tc.tile_pool(name="stage", bufs=2))
    vt_pool = ctx.enter_context(tc.tile_pool(name="vt", bufs=3))
    vbf_pool = ctx.enter_context(tc.tile_pool(name="vbf", bufs=2))
    maskS_pool = ctx.enter_context(tc.tile_pool(name="maskS", bufs=2))
    w1r_pool = ctx.enter_context(tc.tile_pool(name="w1r", bufs=4))
    n1_pool = ctx.enter_context(tc.tile_pool(name="n1", bufs=2))
    out_pool = ctx.enter_context(tc.tile_pool(name="outp", bufs=2))
    rd_pool = ctx.enter_context(tc.tile_pool(name="rd", bufs=2))
    ps_e = ctx.enter_context(tc.tile_pool(name="ps_e", bufs=2, space="PSUM"))
    ps_o = ctx.enter_context(tc.tile_pool(name="ps_o", bufs=1, space="PSUM"))
    ps_d = ctx.enter_context(tc.tile_pool(name="ps_d", bufs=1, space="PSUM"))

    const_pool = ctx.enter_context(tc.tile_pool(name="const", bufs=1))
    onesP = const_pool.tile([P, P], BF16, name="onesP")
    nc.vector.memset(onesP[:], 1.0)

    state = {}
    rep_state = {}

    def prep_rep(r):
        """Once per rep: both batches' masks -> exp bias 2*ln(mask+eps)
        (one Ln activation) and the d-matmul weights w1rep (DVE, tiny,
        gated only on the small mask loads)."""
        W = BS * MCH
        mpt = maskS_pool.tile([P, W], F32, tag="mask_pt", name=f"mpt{r}")
        for b in range(BS):
            nc.sync.dma_start(
                out=mpt[:, b * MCH : (b + 1) * MCH],
                in_=m_d[b, 0, :].rearrange("(j p) -> p j", p=P),
            )
        lnm2 = None
        if EXP_BIAS:
            t1 = maskS_pool.tile([P, W], F32, tag="t1", name=f"t1{r}")
            nc.vector.tensor_scalar_add(t1[:], mpt[:], MEPS)
            lnm = maskS_pool.tile([P, W], F32, tag="lnm", name=f"lnm{r}")
            nc.scalar.activation(out=lnm[:], in_=t1[:], func=LN)
            lnm2 = maskS_pool.tile([P, W], F32, tag="lnm2", name=f"lnm2{r}")
            nc.vector.tensor_scalar_mul(lnm2[:], lnm[:], 2.0)
            t2 = maskS_pool.tile([P, W], F32, tag="t2", name=f"t2{r}")
            nc.vector.tensor_mul(t2[:], t1[:], t1[:])
            t3 = maskS_pool.tile([P, W], F32, tag="t3", name=f"t3{r}")
            nc.vector.reciprocal(out=t3[:], in_=t2[:])
            w1 = maskS_pool.tile([P, W], F32, tag="w1", name=f"w1{r}")
            nc.vector.tensor_scalar_add(w1[:], mpt[:], 1e-9)
            wsrc = maskS_pool.tile([P, W], F32, tag="w1m", name=f"w1m{r}")
            nc.vector.tensor_mul(wsrc[:], w1[:], t3[:])
        else:
            m2 = maskS_pool.tile([P, W], F32, tag="m2", name=f"m2{r}")
            nc.vector.tensor_mul(m2[:], mpt[:], mpt[:])
            rep_m2 = m2
            wsrc = maskS_pool.tile([P, W], F32, tag="w1m", name=f"w1m{r}")
            nc.vector.tensor_scalar_add(wsrc[:], mpt[:], 1e-9)
        w1reps = []
        for b in range(BS):
            w1rep = w1r_pool.tile([P, MCH, P], BF16, tag="w1rep", name=f"w1r{r}_{b}")
            for j in range(MCH):
                nc.vector.tensor_scalar_mul(
                    w1rep[:, j, :], onesP[:], wsrc[:, b * MCH + j : b * MCH + j + 1]
                )
            w1reps.append(w1rep)
        rep_state[r] = {
            "lnm2": lnm2,
            "w1rep": w1reps,
            "m2": None if EXP_BIAS else rep_m2,
        }

    def prep_loads(key, b):
        """DMA loads only (SP queue) — conversions happen in prep_conv.

        V first: its bf16 conversion feeds the vt transposes (also on the
        SP queue, via prep_b) whose wait would otherwise head-of-line-block
        the next batch's loads."""
        st = {}
        for cc in range(CCH):
            vstg = stage_pool.tile([P, L], F32, tag="stage", name=f"stg_v{key}_{cc}")
            nc.sync.dma_start(out=vstg[:], in_=v_d[b, cc * P : (cc + 1) * P, :])
            st[f"stg_v{cc}"] = vstg
        for cc in range(CCH):
            for src, pfx in ((k_d, "k"), (q_d, "q")):
                stg = stage_pool.tile(
                    [P, L], F32, tag="stage", name=f"stg_{pfx}{key}_{cc}"
                )
                nc.sync.dma_start(out=stg[:], in_=src[b, cc * P : (cc + 1) * P, :])
                st[f"stg_{pfx}{cc}"] = stg
        state[key] = st

    def prep_conv(key):
        """All f32->bf16 conversions on Pool (v first: it feeds the
        transposes).  The DVE queue carries nothing load-gated."""
        st = state[key]
        for cc in range(CCH):
            vb = vbf_pool.tile([P, L], BF16, tag="vbf", name=f"vbf{key}_{cc}")
            nc.gpsimd.tensor_copy(vb[:], st[f"stg_v{cc}"][:])
            st[f"v_bf{cc}"] = vb
        q_sb, k_sb = [], []
        for cc in range(CCH):
            kt = qk_pool.tile([P, L], BF16, tag=f"k{cc}", name=f"k{key}_{cc}")
            nc.gpsimd.tensor_copy(kt[:], st[f"stg_k{cc}"][:])
            k_sb.append(kt)
            qt = qk_pool.tile([P, L], BF16, tag=f"q{cc}", name=f"q{key}_{cc}")
            nc.gpsimd.tensor_copy(qt[:], st[f"stg_q{cc}"][:])
            q_sb.append(qt)
        st["q"], st["k"] = q_sb, k_sb

    def prep_b(key):
        """vt transposes (SP queue; vb is ready early by construction),
        plus the in-place mask^2 fold when it isn't riding the exp bias."""
        st = state[key]
        vt = vt_pool.tile([P, MCH, C], BF16, tag="vt", name=f"vt{key}")
        for cc in range(CCH):
            nc.sync.dma_start_transpose(
                out=vt[:, :, cc * P : (cc + 1) * P], in_=st[f"v_bf{cc}"][:]
            )
        if not EXP_BIAS:
            m2 = rep_state[key[0]]["m2"]
            bcol = key[1] * MCH
            for j in range(MCH):
                nc.vector.tensor_scalar_mul(
                    vt[:, j, :], vt[:, j, :], m2[:, bcol + j : bcol + j + 1]
                )
        st["vt"] = vt

    def den(prev):
        """Reciprocal of the PE-computed replicated denominator."""
        pkey, ph, pn1, pd_ps = prev
        rec = rd_pool.tile([P, LH], F32, tag="rec", name=f"rc_{pkey}_{ph}")
        nc.vector.reciprocal_approx_fast(out=rec[:], in_=pd_ps[:])
        return rec

    def jloop(cur, prev, rec):
        """QK + exp + denominator-matmuls for `cur`; AV matmuls of `prev`
        interleaved cg-sequentially with inline normalize+store per cg."""
        n1 = d_ps = None
        if cur is not None:
            key, h = cur
            st = state[key]
            srep = rep_state[key[0]]
            bcol = key[1] * MCH
            w1rep = srep["w1rep"][key[1]]
            lnm2 = srep["lnm2"]
            lq = h * LH
            n1 = n1_pool.tile([P, MCH, LH], BF16, tag="n1", name=f"n1_{key}_{h}")
            d_ps = ps_d.tile([P, LH], F32, tag="D", name=f"d_{key}_{h}")
        av = []
        if prev is not None:
            pkey, ph, pn1, _ = prev
            pst = state[pkey]
            av = [(cg, j, lt) for cg in range(CCH) for j in range(MCH) for lt in range(2)]
        o_ps = {}

        def emit_av(k0, k1):
            for cg, j, lt in av[k0:k1]:
                if lt == 0 and j == 0:
                    o_ps[cg] = ps_o.tile(
                        [P, LH], F32, tag="O", name=f"o_{pkey}_{ph}_{cg}"
                    )
                nc.tensor.matmul(
                    o_ps[cg][:, lt * LT : (lt + 1) * LT],
                    lhsT=pst["vt"][:, j, cg * P : (cg + 1) * P],
                    rhs=pn1[:, j, lt * LT : (lt + 1) * LT],
                    start=(j == 0),
                    stop=(j == MCH - 1),
                )
                if lt == 1 and j == MCH - 1:
                    fin_cg(cg)

        def fin_cg(cg):
            out_t = out_pool.tile([P, LH], F32, tag="out", name=f"ot_{pkey}_{ph}_{cg}")
            nc.vector.tensor_mul(out_t[:], o_ps[cg][:], rec[:])
            nc.sync.dma_start(
                out=o_d[pkey[1], cg * P : (cg + 1) * P, ph * LH : (ph + 1) * LH],
                in_=out_t[:],
            )

        def emit_dmm(j):
            for lt in range(2):
                nc.tensor.matmul(
                    d_ps[:, lt * LT : (lt + 1) * LT],
                    lhsT=w1rep[:, j, :],
                    rhs=n1[:, j, lt * LT : (lt + 1) * LT],
                    start=(j == 0),
                    stop=(j == MCH - 1),
                )

        for j in range(MCH):
            if cur is not None:
                e_ps = ps_e.tile([P, LH], F32, tag="E", name=f"e_{key}_{h}_{j}")
                kj = slice(j * P, (j + 1) * P)
                for lt in range(2):
                    lsl = slice(lq + lt * LT, lq + (lt + 1) * LT)
                    for cc in range(CCH):
                        nc.tensor.matmul(
                            e_ps[:, lt * LT : (lt + 1) * LT],
                            lhsT=st["k"][cc][:, kj],
                            rhs=st["q"][cc][:, lsl],
                            start=(cc == 0),
                            stop=(cc == CCH - 1),
                        )
                if EXP_BIAS:
                    nc.scalar.activation(
                        out=n1[:, j, :],
                        in_=e_ps[:],
                        func=EXP,
                        scale=1.0 / 16.0,
                        bias=lnm2[:, bcol + j : bcol + j + 1],
                    )
                else:
                    nc.scalar.activation(
                        out=n1[:, j, :], in_=e_ps[:], func=EXP, scale=1.0 / 16.0
                    )
            # AV before d(j-1): d's rhs n1(j-1) clears the exp ~700ns after
            # QK(j) starts; AV (prev-job data, always ready) covers that.
            emit_av(4 * j, 4 * (j + 1))
            if cur is not None and j > 0:
                emit_dmm(j - 1)
        if cur is not None:
            emit_dmm(MCH - 1)
        emit_av(4 * MCH, len(av))
        return n1, d_ps

    reps = int(_os.environ.get("BASS_REPS", "1"))
    jobs = [(r, b, h) for r in range(reps) for b in range(BS) for h in range(NH)]
    prep_rep(0)
    prep_loads((0, jobs[0][1]), jobs[0][1])
    prep_conv((0, jobs[0][1]))
    prep_b((0, jobs[0][1]))
    prev = None
    rec = None
    for i, (r, b, h) in enumerate(jobs):
        pending_conv = None
        if h == 0 and i + 2 < len(jobs) and jobs[i + 2][2] == 0:
            nr, nb, _ = jobs[i + 2]
            if nr != r:
                prep_rep(nr)
            prep_loads((nr, nb), nb)
            pending_conv = (nr, nb)
        elif h == 1 and i + 1 < len(jobs) and jobs[i + 1][2] == 0:
            prep_b((jobs[i + 1][0], jobs[i + 1][1]))
        n1, d_ps = jloop(((r, b), h), prev, rec)
        # rec for THIS job emitted immediately: it sits ahead of anything
        # long in the DVE queue, so d_ps (bufs=1) frees before the next
        # jloop's first d-matmul needs the bank.
        prev = ((r, b), h, n1, d_ps)
        rec = den(prev)
        if pending_conv is not None:
            prep_conv(pending_conv)
    jloop(None, prev, rec)


def _build():
    nc = bacc.Bacc(
        "TRN2",
        target_bir_lowering=False,
        debug=False,
        enable_asserts=False,
        num_devices=NCORES,
    )
    q_d = nc.dram_tensor("proj_query", [BS, C, L], F32, kind="ExternalInput")
    k_d = nc.dram_tensor("proj_key", [BS, C, L], F32, kind="ExternalInput")
    v_d = nc.dram_tensor("proj_val", [BS, C, L], F32, kind="ExternalInput")
    m_d = nc.dram_tensor("padding_mask", [BS, 1, L], F32, kind="ExternalInput")
    o_d = nc.dram_tensor("out", [BS, C, L], F32, kind="ExternalOutput")

    from contextlib import ExitStack

    with tile.TileContext(nc) as tc:
        with ExitStack() as ctx:
            _emit(ctx, tc, q_d.ap(), k_d.ap(), v_d.ap(), m_d.ap(), o_d.ap())
    nc.compile()
    return nc


_cached_nc = None


def get_nc():
    global _cached_nc
    if _cached_nc is None:
        _cached_nc = _build()
    return _cached_nc


def make_in_maps(proj_query, proj_key, proj_val, padding_mask):
    q = np.ascontiguousarray(np.asarray(proj_query, dtype=np.float32))
    k = np.ascontiguousarray(np.asarray(proj_key, dtype=np.float32))
    v = np.ascontiguousarray(np.asarray(proj_val, dtype=np.float32))
    m = np.ascontiguousarray(np.asarray(padding_mask, dtype=np.float32))
    assert q.shape == (B, C, L) and m.shape == (B, 1, L)
    in_maps = []
    for i in range(NCORES):
        sl = slice(i * BS, (i + 1) * BS)
        in_maps.append(
            {
                "proj_query": np.ascontiguousarray(q[sl]),
                "proj_key": np.ascontiguousarray(k[sl]),
                "proj_val": np.ascontiguousarray(v[sl]),
                "padding_mask": np.ascontiguousarray(m[sl]),
            }
        )
    return in_maps


def kernel(proj_query, proj_key, proj_val, padding_mask):
    nc = get_nc()
    in_maps = make_in_maps(proj_query, proj_key, proj_val, padding_mask)
    res = run_bass_kernel_spmd(nc, in_maps, core_ids=list(range(NCORES)))
    return np.concatenate([res.results[i]["out"] for i in range(NCORES)], axis=0)


# revision 24
# speedup vs baseline: 1.2167x; 1.2167x over previous
"""Trainium2 Bass kernel for nn_AttentionHelper (sparse_attention) — v8.

Math (per batch b):
    nt[m,l] = exp(E[m,l]/16 + 2*ln(mask[m]+eps))   (= mask^2 * exp(E/16))
    d[l]    = sum_m w1m[m] * nt[m,l],  w1m = (mask+1e-9)/(mask+eps)^2
    out     = sum_m V[c,m] * nt[m,l] / d[l]

HW-informed design (microbenchmarked): bf16 matmuls run at ~107ns per
[128x128]x[128x512] (dual-pumped) when the PE stays continuously busy;
fp8 DoubleRow measured SLOWER (144ns) than bf16 on HW, so everything
stays bf16.  The design goal is keeping the PE stream gap-free so the
p-state stays at max:
  - denominator on the PE: d_ps[p,l] += w1rep[:,j,:]^T @ nt[:,j,:] with
    w1rep a broadcast-replicated weight column (reduces over m AND
    broadcasts d to all partitions in one op).
  - mask^2 folded into the exp's per-partition BIAS (2*ln(mask+eps)),
    computed once per rep by a single Ln activation over both batches'
    masks — removes the 16 post-transpose vt-fold DVE muls whose
    transpose-gated wait serialized the DVE queue at rep boundaries.
  - per-j PE order QK -> AV(prev) -> d(j-1): the d-matmul reads n1(j-1)
    which only clears the exp ~678ns after QK(j) starts; AV first covers
    that latency, d never stalls the PE.
  - ALL f32->bf16 conversions (q,k,v) on Pool/gpsimd: the DVE queue
    keeps only short, early-ready ops (w1rep broadcasts, normalize muls,
    reciprocal) so fin/rec are never head-of-line blocked by load-gated
    conversions.
  - loads V-first + vb converted first, so the vt transposes (SP queue)
    never block the next batch's loads.
  - rec emitted immediately after each jloop: d_ps (bufs=1) frees before
    the next jloop's first d-matmul needs the bank.
  - AV runs cg-sequential (cg0 done mid-loop, normalized+stored while
    cg1 accumulates) so PSUM fits: e_ps 2x2 + o_ps 1x2 + d_ps 1x2 = 8.

Pipeline: flat (rep, batch, half) jobs; prev carried across batch AND rep
boundaries; all DMA (loads, stores, transposes) on the SP HWDGE queue.
"""

import numpy as np

import concourse.bacc as bacc
import concourse.bass as bass
import concourse.tile as tile
from concourse import mybir
from concourse.bass_utils import run_bass_kernel_spmd

B, C, L = 16, 256, 2048
NCORES = 8
BS = B // NCORES
P = 128
CCH = C // P
MCH = L // P
NH = 2
LH = L // NH
LT = 512
F32 = mybir.dt.float32
BF16 = mybir.dt.bfloat16
import os as _os

EXP = mybir.ActivationFunctionType.Exp
LN = mybir.ActivationFunctionType.Ln
MEPS = 3e-5
# A/B toggles (HW-measured): exp-bias mask fold vs DVE vt-fold muls
EXP_BIAS = _os.environ.get("BASS_EXP_BIAS", "1") == "1"
# q-copies on Pool measured slow on HW (Q7 software engine); keep q on DVE
Q_ON_POOL = _os.environ.get("BASS_Q_POOL", "0") == "1"


def _emit(ctx, tc, q_d, k_d, v_d, m_d, o_d):
    nc = tc.nc

    qk_pool = ctx.enter_context(tc.tile_pool(name="qk", bufs=2))
    stage_pool = ctx.enter_context(tc.tile_pool(name="stage", bufs=2))
    vt_pool = ctx.enter_context(tc.tile_pool(name="vt", bufs=3))
    vbf_pool = ctx.enter_context(tc.tile_pool(name="vbf", bufs=2))
    maskS_pool = ctx.enter_context(tc.tile_pool(name="maskS", bufs=2))
    w1r_pool = ctx.enter_context(tc.tile_pool(name="w1r", bufs=4))
    n1_pool = ctx.enter_context(tc.tile_pool(name="n1", bufs=2))
    out_pool = ctx.enter_context(tc.tile_pool(name="outp", bufs=2))
    rd_pool = ctx.enter_context(tc.tile_pool(name="rd", bufs=2))
    ps_e = ctx.enter_context(tc.tile_pool(name="ps_e", bufs=2, space="PSUM"))
    ps_o = ctx.enter_context(tc.tile_pool(name="ps_o", bufs=1, space="PSUM"))
    ps_d = ctx.enter_context(tc.tile_pool(name="ps_d", bufs=1, space="PSUM"))

    const_pool = ctx.enter_context(tc.tile_pool(name="const", bufs=1))
    onesP = const_pool.tile([P, P], BF16, name="onesP")
    nc.vector.memset(onesP[:], 1.0)

    state = {}
    rep_state = {}

    def prep_rep(r):
        """Once per rep: both batches' masks -> exp bias 2*ln(mask+eps)
        (one Ln activation) and the d-matmul weights w1rep (DVE, tiny,
        gated only on the small mask loads)."""
        W = BS * MCH
        mpt = maskS_pool.tile([P, W], F32, tag="mask_pt", name=f"mpt{r}")
        for b in range(BS):
            # ACT HWDGE queue: data-independent load, never queues behind
            # stores on the SP queue, so the Ln below is ready early and
            # can't head-of-line-block the exp stream.
            nc.scalar.dma_start(
                out=mpt[:, b * MCH : (b + 1) * MCH],
                in_=m_d[b, 0, :].rearrange("(j p) -> p j", p=P),
            )
        lnm2 = None
        if EXP_BIAS:
            t1 = maskS_pool.tile([P, W], F32, tag="t1", name=f"t1{r}")
            nc.vector.tensor_scalar_add(t1[:], mpt[:], MEPS)
            lnm = maskS_pool.tile([P, W], F32, tag="lnm", name=f"lnm{r}")
            nc.scalar.activation(out=lnm[:], in_=t1[:], func=LN)
            lnm2 = maskS_pool.tile([P, W], F32, tag="lnm2", name=f"lnm2{r}")
            nc.vector.tensor_scalar_mul(lnm2[:], lnm[:], 2.0)
            t2 = maskS_pool.tile([P, W], F32, tag="t2", name=f"t2{r}")
            nc.vector.tensor_mul(t2[:], t1[:], t1[:])
            t3 = maskS_pool.tile([P, W], F32, tag="t3", name=f"t3{r}")
            nc.vector.reciprocal(out=t3[:], in_=t2[:])
            w1 = maskS_pool.tile([P, W], F32, tag="w1", name=f"w1{r}")
            nc.vector.tensor_scalar_add(w1[:], mpt[:], 1e-9)
            wsrc = maskS_pool.tile([P, W], F32, tag="w1m", name=f"w1m{r}")
            nc.vector.tensor_mul(wsrc[:], w1[:], t3[:])
        else:
            m2 = maskS_pool.tile([P, W], F32, tag="m2", name=f"m2{r}")
            nc.vector.tensor_mul(m2[:], mpt[:], mpt[:])
            rep_m2 = m2
            wsrc = maskS_pool.tile([P, W], F32, tag="w1m", name=f"w1m{r}")
            nc.vector.tensor_scalar_add(wsrc[:], mpt[:], 1e-9)
        w1reps = []
        for b in range(BS):
            w1rep = w1r_pool.tile([P, MCH, P], BF16, tag="w1rep", name=f"w1r{r}_{b}")
            for j in range(MCH):
                nc.vector.tensor_scalar_mul(
                    w1rep[:, j, :], onesP[:], wsrc[:, b * MCH + j : b * MCH + j + 1]
                )
            w1reps.append(w1rep)
        rep_state[r] = {
            "lnm2": lnm2,
            "w1rep": w1reps,
            "m2": None if EXP_BIAS else rep_m2,
        }

    def prep_loads(key, b):
        """DMA loads only (SP queue) — conversions happen in prep_conv.

        V first: its bf16 conversion feeds the vt transposes (also on the
        SP queue, via prep_b) whose wait would otherwise head-of-line-block
        the next batch's loads."""
        st = {}
        for cc in range(CCH):
            vstg = stage_pool.tile([P, L], F32, tag="stage", name=f"stg_v{key}_{cc}")
            nc.sync.dma_start(out=vstg[:], in_=v_d[b, cc * P : (cc + 1) * P, :])
            st[f"stg_v{cc}"] = vstg
        for cc in range(CCH):
            for src, pfx in ((k_d, "k"), (q_d, "q")):
                stg = stage_pool.tile(
                    [P, L], F32, tag="stage", name=f"stg_{pfx}{key}_{cc}"
                )
                nc.sync.dma_start(out=stg[:], in_=src[b, cc * P : (cc + 1) * P, :])
                st[f"stg_{pfx}{cc}"] = stg
        state[key] = st

    def prep_conv(key):
        """All f32->bf16 conversions on Pool (v first: it feeds the
        transposes).  The DVE queue carries nothing load-gated."""
        st = state[key]
        for cc in range(CCH):
            vb = vbf_pool.tile([P, L], BF16, tag="vbf", name=f"vbf{key}_{cc}")
            nc.gpsimd.tensor_copy(vb[:], st[f"stg_v{cc}"][:])
            st[f"v_bf{cc}"] = vb
        q_sb, k_sb = [], []
        for cc in range(CCH):
            kt = qk_pool.tile([P, L], BF16, tag=f"k{cc}", name=f"k{key}_{cc}")
            nc.gpsimd.tensor_copy(kt[:], st[f"stg_k{cc}"][:])
            k_sb.append(kt)
            qt = qk_pool.tile([P, L], BF16, tag=f"q{cc}", name=f"q{key}_{cc}")
            if Q_ON_POOL:
                nc.gpsimd.tensor_copy(qt[:], st[f"stg_q{cc}"][:])
            else:
                nc.vector.tensor_copy(qt[:], st[f"stg_q{cc}"][:])
            q_sb.append(qt)
        st["q"], st["k"] = q_sb, k_sb

    def prep_b(key):
        """vt transposes (SP queue; vb is ready early by construction),
        plus the in-place mask^2 fold when it isn't riding the exp bias."""
        st = state[key]
        vt = vt_pool.tile([P, MCH, C], BF16, tag="vt", name=f"vt{key}")
        for cc in range(CCH):
            nc.sync.dma_start_transpose(
                out=vt[:, :, cc * P : (cc + 1) * P], in_=st[f"v_bf{cc}"][:]
            )
        if not EXP_BIAS:
            m2 = rep_state[key[0]]["m2"]
            bcol = key[1] * MCH
            for j in range(MCH):
                nc.vector.tensor_scalar_mul(
                    vt[:, j, :], vt[:, j, :], m2[:, bcol + j : bcol + j + 1]
                )
        st["vt"] = vt

    def den(prev):
        """Reciprocal of the PE-computed replicated denominator."""
        pkey, ph, pn1, pd_ps = prev
        rec = rd_pool.tile([P, LH], F32, tag="rec", name=f"rc_{pkey}_{ph}")
        nc.vector.reciprocal_approx_fast(out=rec[:], in_=pd_ps[:])
        return rec

    def jloop(cur, prev, rec):
        """QK + exp + denominator-matmuls for `cur`; AV matmuls of `prev`
        interleaved cg-sequentially with inline normalize+store per cg."""
        n1 = d_ps = None
        if cur is not None:
            key, h = cur
            st = state[key]
            srep = rep_state[key[0]]
            bcol = key[1] * MCH
            w1rep = srep["w1rep"][key[1]]
            lnm2 = srep["lnm2"]
            lq = h * LH
            n1 = n1_pool.tile([P, MCH, LH], BF16, tag="n1", name=f"n1_{key}_{h}")
            d_ps = ps_d.tile([P, LH], F32, tag="D", name=f"d_{key}_{h}")
        av = []
        if prev is not None:
            pkey, ph, pn1, _ = prev
            pst = state[pkey]
            av = [(cg, j, lt) for cg in range(CCH) for j in range(MCH) for lt in range(2)]
        o_ps = {}

        def emit_av(k0, k1):
            for cg, j, lt in av[k0:k1]:
                if lt == 0 and j == 0:
                    o_ps[cg] = ps_o.tile(
                        [P, LH], F32, tag="O", name=f"o_{pkey}_{ph}_{cg}"
                    )
                nc.tensor.matmul(
                    o_ps[cg][:, lt * LT : (lt + 1) * LT],
                    lhsT=pst["vt"][:, j, cg * P : (cg + 1) * P],
                    rhs=pn1[:, j, lt * LT : (lt + 1) * LT],
                    start=(j == 0),
                    stop=(j == MCH - 1),
                )
                if lt == 1 and j == MCH - 1:
                    fin_cg(cg)

        def fin_cg(cg):
            out_t = out_pool.tile([P, LH], F32, tag="out", name=f"ot_{pkey}_{ph}_{cg}")
            nc.vector.tensor_mul(out_t[:], o_ps[cg][:], rec[:])
            nc.sync.dma_start(
                out=o_d[pkey[1], cg * P : (cg + 1) * P, ph * LH : (ph + 1) * LH],
                in_=out_t[:],
            )

        def emit_dmm(j):
            for lt in range(2):
                nc.tensor.matmul(
                    d_ps[:, lt * LT : (lt + 1) * LT],
                    lhsT=w1rep[:, j, :],
                    rhs=n1[:, j, lt * LT : (lt + 1) * LT],
                    start=(j == 0),
                    stop=(j == MCH - 1),
                )

        for j in range(MCH):
            if cur is not None:
                e_ps = ps_e.tile([P, LH], F32, tag="E", name=f"e_{key}_{h}_{j}")
                kj = slice(j * P, (j + 1) * P)
                for lt in range(2):
                    lsl = slice(lq + lt * LT, lq + (lt + 1) * LT)
                    for cc in range(CCH):
                        nc.tensor.matmul(
                            e_ps[:, lt * LT : (lt + 1) * LT],
                            lhsT=st["k"][cc][:, kj],
                            rhs=st["q"][cc][:, lsl],
                            start=(cc == 0),
                            stop=(cc == CCH - 1),
                        )
                if EXP_BIAS:
                    nc.scalar.activation(
                        out=n1[:, j, :],
                        in_=e_ps[:],
                        func=EXP,
                        scale=1.0 / 16.0,
                        bias=lnm2[:, bcol + j : bcol + j + 1],
                    )
                else:
                    nc.scalar.activation(
                        out=n1[:, j, :], in_=e_ps[:], func=EXP, scale=1.0 / 16.0
                    )
            # AV before d(j-1): d's rhs n1(j-1) clears the exp ~700ns after
            # QK(j) starts; AV (prev-job data, always ready) covers that.
            emit_av(4 * j, 4 * (j + 1))
            if cur is not None and j > 0:
                emit_dmm(j - 1)
        if cur is not None:
            emit_dmm(MCH - 1)
        emit_av(4 * MCH, len(av))
        return n1, d_ps

    reps = int(_os.environ.get("BASS_REPS", "1"))
    jobs = [(r, b, h) for r in range(reps) for b in range(BS) for h in range(NH)]
    prep_rep(0)
    prep_loads((0, jobs[0][1]), jobs[0][1])
    prep_conv((0, jobs[0][1]))
    prep_b((0, jobs[0][1]))
    prev = None
    rec = None
    for i, (r, b, h) in enumerate(jobs):
        pending_conv = None
        if h == 0 and i + 2 < len(jobs) and jobs[i + 2][2] == 0:
            nr, nb, _ = jobs[i + 2]
            if nr != r:
                prep_rep(nr)
            prep_loads((nr, nb), nb)
            pending_conv = (nr, nb)
        elif h == 1 and i + 1 < len(jobs) and jobs[i + 1][2] == 0:
            prep_b((jobs[i + 1][0], jobs[i + 1][1]))
        n1, d_ps = jloop(((r, b), h), prev, rec)
        # rec for THIS job emitted immediately: it sits ahead of anything
        # long in the DVE queue, so d_ps (bufs=1) frees before the next
        # jloop's first d-matmul needs the bank.
        prev = ((r, b), h, n1, d_ps)
        rec = den(prev)
        if pending_conv is not None:
            prep_conv(pending_conv)
    jloop(None, prev, rec)


def _build():
    nc = bacc.Bacc(
        "TRN2",
        target_bir_lowering=False,
        debug=False,
        enable_asserts=False,
        num_devices=NCORES,
    )
    q_d = nc.dram_tensor("proj_query", [BS, C, L], F32, kind="ExternalInput")
    k_d = nc.dram_tensor("proj_key", [BS, C, L], F32, kind="ExternalInput")
    v_d = nc.dram_tensor("proj_val", [BS, C, L], F32, kind="ExternalInput")
    m_d = nc.dram_tensor("padding_mask", [BS, 1, L], F32, kind="ExternalInput")
    o_d = nc.dram_tensor("out", [BS, C, L], F32, kind="ExternalOutput")

    from contextlib import ExitStack

    with tile.TileContext(nc) as tc:
        with ExitStack() as ctx:
            _emit(ctx, tc, q_d.ap(), k_d.ap(), v_d.ap(), m_d.ap(), o_d.ap())
    nc.compile()
    return nc


_cached_nc = None


def get_nc():
    global _cached_nc
    if _cached_nc is None:
        _cached_nc = _build()
    return _cached_nc


def make_in_maps(proj_query, proj_key, proj_val, padding_mask):
    q = np.ascontiguousarray(np.asarray(proj_query, dtype=np.float32))
    k = np.ascontiguousarray(np.asarray(proj_key, dtype=np.float32))
    v = np.ascontiguousarray(np.asarray(proj_val, dtype=np.float32))
    m = np.ascontiguousarray(np.asarray(padding_mask, dtype=np.float32))
    assert q.shape == (B, C, L) and m.shape == (B, 1, L)
    in_maps = []
    for i in range(NCORES):
        sl = slice(i * BS, (i + 1) * BS)
        in_maps.append(
            {
                "proj_query": np.ascontiguousarray(q[sl]),
                "proj_key": np.ascontiguousarray(k[sl]),
                "proj_val": np.ascontiguousarray(v[sl]),
                "padding_mask": np.ascontiguousarray(m[sl]),
            }
        )
    return in_maps


def kernel(proj_query, proj_key, proj_val, padding_mask):
    nc = get_nc()
    in_maps = make_in_maps(proj_query, proj_key, proj_val, padding_mask)
    res = run_bass_kernel_spmd(nc, in_maps, core_ids=list(range(NCORES)))
    return np.concatenate([res.results[i]["out"] for i in range(NCORES)], axis=0)
